# revision 4
# baseline (speedup 1.0000x reference)
"""Trainium2 Bass kernel for the SCAN cross-attention contrastive loss.

Math (validated against the reference, exact in fp64):
  For caption c with words zeroed beyond its true length:
    A[w, ir]   = <recipes[c, w, :], images_flat[ir, :]>           (raw attention)
    L          = leaky_relu(A, 0.1)       (zero rows stay zero)
    rinv9[ir]  = 9 / ||L[:, ir]||
    E          = exp(L * rinv9)           (softmax numerator; the denominator
                                           cancels in the cosine below)
    num[ir]    = sum_w E * A              (invalid rows: E*0 = 0)
    u2[ir]     = sum_w E * (G @ E),  G = R_c R_c^T  (Gram; zero rows kill pad E)
    rs[ir]     = num / (||img_ir|| * sqrt(u2))      (= row_sim of the reference)
    score[c,i] = sum_{r in image i} exp(6 * rs)     (log/6 applied on host)
  Final hinge-loss reduction over the 96x96 score matrix is done on host (tiny).

Sharding: captions sorted by length, dealt round-robin to 8 cores (slot s of
core k = sorted[s*8+k]) so every core shares the same per-slot padded length,
then slots are FFD-bin-packed into <=128-partition groups. One SPMD program;
per-core data (packed recipes, Gram blocks, masks) differs.
"""

import sys

sys.path.insert(0, "/opt/trn_rl_repo")

import numpy as np

I, R, D = 96, 36, 256
C, W = 96, 48
IR = I * R  # 3456
N_CORES = 8
CPC = C // N_CORES  # captions per core = 12
CHUNK = 432  # IR columns per iteration = 12 images = one PSUM bank
N_CHUNKS = IR // CHUNK  # 8
IMG_PER_CHUNK = CHUNK // R  # 12

_CACHE = {}


def _plan_groups(cap_lens):
    """Slot lengths (max over cores per round-robin slot) + FFD packing."""
    order = np.argsort(-cap_lens, kind="stable")  # longest first
    slot_len = [int(cap_lens[order[s * N_CORES]]) for s in range(CPC)]
    bins = []  # [rows_used, [slot indices]]
    for s in range(CPC):
        ln = slot_len[s]
        for b in bins:
            if b[0] + ln <= 128:
                b[1].append(s)
                b[0] += ln
                break
        else:
            bins.append([ln, [s]])
    groups = []
    for rows, slots in bins:
        offs = []
        off = 0
        for s in slots:
            offs.append(off)
            off += slot_len[s]
        groups.append({"slots": slots, "offs": offs, "P": rows})
    return order, slot_len, groups


def _build_program(cap_lens):
    import concourse.bacc as bacc
    import concourse.mybir as mybir
    from concourse.tile import TileContext

    fp32 = mybir.dt.float32
    f32r = mybir.dt.float32r
    bf16 = mybir.dt.bfloat16
    ACT = mybir.ActivationFunctionType
    ALU = mybir.AluOpType
    AX = mybir.AxisListType

    order, slot_len, groups = _plan_groups(cap_lens)
    NG = len(groups)

    nc = bacc.Bacc("TRN2", target_bir_lowering=False, debug=False,
                   num_devices=N_CORES)

    imagesT_d = nc.dram_tensor("imagesT", [2, 128, IR], f32r, kind="ExternalInput")
    n1sq_d = nc.dram_tensor("n1sq", [CPC, IR], fp32, kind="ExternalInput")
    recT_d, G_d, colsum_d, bcast_d, capmask_d = [], [], [], [], []
    for g, gr in enumerate(groups):
        P = gr["P"]
        ncg = len(gr["slots"])
        recT_d.append(nc.dram_tensor(f"recT{g}", [2, 128, P], f32r,
                                     kind="ExternalInput"))
        G_d.append(nc.dram_tensor(f"G{g}", [P, P], bf16, kind="ExternalInput"))
        colsum_d.append(nc.dram_tensor(f"cs{g}", [P, ncg], bf16,
                                       kind="ExternalInput"))
        bcast_d.append(nc.dram_tensor(f"bc{g}", [ncg, P], bf16,
                                      kind="ExternalInput"))
        capmask_d.append(nc.dram_tensor(f"cm{g}", [P, CPC], bf16,
                                        kind="ExternalInput"))
    out_d = nc.dram_tensor("scores", [CPC, I], fp32, kind="ExternalOutput")

    with TileContext(nc) as tc:
        with (
            tc.tile_pool(name="const", bufs=1) as cpool,
            tc.tile_pool(name="work", bufs=3) as wpool,
            tc.tile_pool(name="small", bufs=2) as spool,
            tc.tile_pool(name="psA", bufs=2, space="PSUM") as psA,
            tc.tile_pool(name="psF", bufs=2, space="PSUM") as psF,
            tc.tile_pool(name="psS", bufs=1, space="PSUM") as psS,
            tc.tile_pool(name="psAcc", bufs=1, space="PSUM") as psAcc,
        ):
            # ---- resident constants ----
            imgT = cpool.tile([128, 2 * IR], f32r, tag="imgT")
            for kc in range(2):
                nc.gpsimd.dma_start(out=imgT[:, kc * IR:(kc + 1) * IR],
                                    in_=imagesT_d[kc, :, :])
            n1sq = cpool.tile([CPC, IR], fp32, tag="n1sq")
            nc.gpsimd.dma_start(out=n1sq[:, :], in_=n1sq_d[:, :])
            recT, Gt, cst, bct, cmt = [], [], [], [], []
            for g, gr in enumerate(groups):
                P = gr["P"]
                ncg = len(gr["slots"])
                rt = cpool.tile([128, 2 * P], f32r, tag=f"recT{g}")
                for kc in range(2):
                    nc.gpsimd.dma_start(out=rt[:, kc * P:(kc + 1) * P],
                                        in_=recT_d[g][kc, :, :])
                recT.append(rt)
                gt = cpool.tile([P, P], bf16, tag=f"G{g}")
                nc.gpsimd.dma_start(out=gt[:, :], in_=G_d[g][:, :])
                Gt.append(gt)
                ct = cpool.tile([P, ncg], bf16, tag=f"cs{g}")
                nc.gpsimd.dma_start(out=ct[:, :], in_=colsum_d[g][:, :])
                cst.append(ct)
                bt = cpool.tile([ncg, P], bf16, tag=f"bc{g}")
                nc.gpsimd.dma_start(out=bt[:, :], in_=bcast_d[g][:, :])
                bct.append(bt)
                mt = cpool.tile([P, CPC], bf16, tag=f"cm{g}")
                nc.gpsimd.dma_start(out=mt[:, :], in_=capmask_d[g][:, :])
                cmt.append(mt)
            scores = cpool.tile([CPC, I], fp32, tag="scores")

            for j in range(N_CHUNKS):
                j0 = j * CHUNK
                num_ps = psAcc.tile([CPC, CHUNK], fp32, tag="num")
                u2_ps = psAcc.tile([CPC, CHUNK], fp32, tag="u2")
                for g, gr in enumerate(groups):
                    P = gr["P"]
                    ncg = len(gr["slots"])
                    first, last = g == 0, g == NG - 1
                    # A_raw = recT.T @ imagesT  (f32r)
                    A_ps = psA.tile([128, CHUNK], fp32, tag="A")
                    for kc in range(2):
                        nc.tensor.matmul(
                            A_ps[:P, :],
                            recT[g][:, kc * P:(kc + 1) * P],
                            imgT[:, kc * IR + j0:kc * IR + j0 + CHUNK],
                            start=(kc == 0), stop=(kc == 1))
                    # leaky: L = prelu(A, alpha=0.1)
                    L = wpool.tile([128, CHUNK], fp32, tag="L")
                    nc.scalar.activation(L[:P, :], A_ps[:P, :], ACT.Prelu,
                                         alpha=0.1)
                    # Lsq (bf16, gpsimd: SBUF only)
                    Lsq = wpool.tile([128, CHUNK], bf16, tag="Lsq")
                    nc.gpsimd.tensor_mul(Lsq[:P, :], L[:P, :], L[:P, :])
                    # nrm2 = per-caption column sums of Lsq
                    n_ps = psS.tile([16, CHUNK], fp32, tag="nrm2")
                    nc.tensor.matmul(n_ps[:ncg, :], cst[g][:, :], Lsq[:P, :],
                                     start=True, stop=True)
                    # rinv9 = 9/sqrt(nrm2) = exp(-0.5*ln(nrm2/81))  (bf16:
                    # scale error cancels in the cosine; tiny temperature shift)
                    lnt = spool.tile([16, CHUNK], fp32, tag="lnt")
                    nc.scalar.activation(lnt[:ncg, :], n_ps[:ncg, :], ACT.Ln,
                                         scale=1.0 / 81.0)
                    rinv = spool.tile([16, CHUNK], bf16, tag="rinv")
                    nc.scalar.activation(rinv[:ncg, :], lnt[:ncg, :], ACT.Exp,
                                         scale=-0.5)
                    # broadcast rinv9 back to word rows
                    rb_ps = psS.tile([128, CHUNK], fp32, tag="rb")
                    nc.tensor.matmul(rb_ps[:P, :], bct[g][:, :], rinv[:ncg, :],
                                     start=True, stop=True)
                    # T = L * rinv_b ; E = exp(T) (bf16)
                    T = wpool.tile([128, CHUNK], fp32, tag="T")
                    nc.vector.tensor_mul(T[:P, :], L[:P, :], rb_ps[:P, :])
                    E = wpool.tile([128, CHUNK], bf16, tag="E")
                    nc.scalar.activation(E[:P, :], T[:P, :], ACT.Exp)
                    # F = G @ E
                    F_ps = psF.tile([128, CHUNK], fp32, tag="F")
                    nc.tensor.matmul(F_ps[:P, :], Gt[g][:, :], E[:P, :],
                                     start=True, stop=True)
                    # P1 = E*A, P2 = E*F (bf16)
                    P1 = wpool.tile([128, CHUNK], bf16, tag="P1")
                    nc.vector.tensor_mul(P1[:P, :], E[:P, :], A_ps[:P, :])
                    P2 = wpool.tile([128, CHUNK], bf16, tag="P2")
                    nc.vector.tensor_mul(P2[:P, :], E[:P, :], F_ps[:P, :])
                    # num += capmask.T @ P1 ; u2 += capmask.T @ P2
                    nc.tensor.matmul(num_ps[:, :], cmt[g][:, :], P1[:P, :],
                                     start=first, stop=last)
                    nc.tensor.matmul(u2_ps[:, :], cmt[g][:, :], P2[:P, :],
                                     start=first, stop=last)
                # rs = num * rsqrt(u2 * n1sq) ; sumexp_r exp(6*rs) per image
                w_t = spool.tile([CPC, CHUNK], fp32, tag="w")
                nc.vector.tensor_mul(w_t[:, :], u2_ps[:, :],
                                     n1sq[:, j0:j0 + CHUNK])
                lw_t = spool.tile([CPC, CHUNK], fp32, tag="lw")
                nc.scalar.activation(lw_t[:, :], w_t[:, :], ACT.Ln)
                q_t = spool.tile([CPC, CHUNK], fp32, tag="q")
                nc.scalar.activation(q_t[:, :], lw_t[:, :], ACT.Exp, scale=-0.5)
                rs_t = spool.tile([CPC, CHUNK], fp32, tag="rs")
                nc.vector.tensor_mul(rs_t[:, :], num_ps[:, :], q_t[:, :])
                e6_t = spool.tile([CPC, CHUNK], fp32, tag="e6")
                nc.scalar.activation(e6_t[:, :], rs_t[:, :], ACT.Exp, scale=6.0)
                nc.vector.tensor_reduce(
                    scores[:, j * IMG_PER_CHUNK:(j + 1) * IMG_PER_CHUNK],
                    e6_t[:, :].rearrange("p (i r) -> p i r", r=R),
                    axis=AX.X, op=ALU.add)
            nc.gpsimd.dma_start(out=out_d[:, :], in_=scores[:, :])
    nc.compile()
    return nc, order, slot_len, groups


def _host_inputs(images, recipes, cap_lens, order, slot_len, groups):
    import ml_dtypes

    imgf = images.reshape(IR, D).astype(np.float32)
    imagesT = np.ascontiguousarray(imgf.T).reshape(2, 128, IR)
    n1sq = (imgf.astype(np.float64) ** 2).sum(axis=1).astype(np.float32)
    n1sq_rep = np.ascontiguousarray(np.broadcast_to(n1sq, (CPC, IR)))

    in_maps = []
    for k in range(N_CORES):
        m = {"imagesT": imagesT, "n1sq": n1sq_rep}
        for g, gr in enumerate(groups):
            P = gr["P"]
            ncg = len(gr["slots"])
            Rg = np.zeros((P, D), np.float32)  # packed, zero-padded recipes
            cs = np.zeros((P, ncg), np.float32)
            bc = np.zeros((ncg, P), np.float32)
            cm = np.zeros((P, CPC), np.float32)
            Gm = np.zeros((P, P), np.float32)
            for li, (s, off) in enumerate(zip(gr["slots"], gr["offs"])):
                cap = int(order[s * N_CORES + k])
                ln = int(cap_lens[cap])
                lp = slot_len[s]
                rws = recipes[cap, :ln, :].astype(np.float32)
                Rg[off:off + ln, :] = rws
                Gm[off:off + ln, off:off + ln] = rws @ rws.T
                cs[off:off + lp, li] = 1.0
                bc[li, off:off + lp] = 1.0
                cm[off:off + lp, s] = 1.0
            m[f"recT{g}"] = np.ascontiguousarray(Rg.T).reshape(2, 128, P)
            m[f"G{g}"] = Gm.astype(ml_dtypes.bfloat16)
            m[f"cs{g}"] = cs.astype(ml_dtypes.bfloat16)
            m[f"bc{g}"] = bc.astype(ml_dtypes.bfloat16)
            m[f"cm{g}"] = cm.astype(ml_dtypes.bfloat16)
        in_maps.append(m)
    return in_maps


def run_sharded(images, recipes, cap_lens, **spmd_kwargs):
    """Compile (cached), run on 8 cores, return (sumexp (C, I) fp64, results)."""
    from concourse.bass_utils import run_bass_kernel_spmd

    cap_lens = np.asarray(cap_lens).astype(np.int32)
    key = cap_lens.tobytes()
    if key not in _CACHE:
        _CACHE[key] = _build_program(cap_lens)
    nc, order, slot_len, groups = _CACHE[key]

    in_maps = _host_inputs(np.asarray(images), np.asarray(recipes), cap_lens,
                           order, slot_len, groups)
    res = run_bass_kernel_spmd(nc, in_maps, list(range(N_CORES)), **spmd_kwargs)

    sumexp = np.zeros((C, I), np.float64)
    for k in range(N_CORES):
        sc = res.results[k]["scores"].astype(np.float64)  # [CPC, I]
        for s in range(CPC):
            sumexp[int(order[s * N_CORES + k])] = sc[s]
    return sumexp, res


def kernel(images, recipes, cap_lens):
    sumexp, _ = run_sharded(images, recipes, cap_lens)
    S = (np.log(sumexp) / 6.0).T  # (I, C)
    diag = np.diag(S)
    eye = np.eye(I, dtype=bool)
    ci = np.where(eye, 0.0, np.maximum(0.2 + S - diag[None, :], 0.0))
    cr = np.where(eye, 0.0, np.maximum(0.2 + S - diag[:, None], 0.0))
    return np.float32(ci.sum() + cr.sum())


# revision 9
# speedup vs baseline: 1.6098x; 1.6098x over previous
"""Trainium2 Bass kernel for the SCAN cross-attention contrastive loss.

Math (validated against the reference, exact in fp64):
  For caption c with words zeroed beyond its true length:
    A[w, ir]   = <recipes[c, w, :], images_flat[ir, :]>           (raw attention)
    L          = leaky_relu(A, 0.1)       (zero rows stay zero)
    rinv9[ir]  = 9 / ||L[:, ir]||
    E          = exp(L * rinv9)           (softmax numerator; the denominator
                                           cancels in the cosine below)
    num[ir]    = sum_w E * A              (invalid rows: E*0 = 0)
    u2[ir]     = sum_w E * (G @ E),  G = R_c R_c^T  (Gram; zero rows kill pad E)
    rs[ir]     = num / (||img_ir|| * sqrt(u2))      (= row_sim of the reference)
    score[c,i] = sum_{r in image i} exp(6 * rs)     (log/6 applied on host)
  Final hinge-loss reduction over the 96x96 score matrix is done on host (tiny).

Sharding: captions sorted by length, dealt round-robin to 8 cores (slot s of
core k = sorted[s*8+k]) so every core shares the same per-slot padded length,
then slots are FFD-bin-packed into <=128-partition groups. One SPMD program;
per-core data (packed recipes, Gram blocks, masks) differs.
"""

import sys

sys.path.insert(0, "/opt/trn_rl_repo")

import numpy as np

I, R, D = 96, 36, 256
C, W = 96, 48
IR = I * R  # 3456
N_CORES = 8
CPC = C // N_CORES  # captions per core = 12
CHUNK = 432  # IR columns per iteration = 12 images = one PSUM bank
N_CHUNKS = IR // CHUNK  # 8
IMG_PER_CHUNK = CHUNK // R  # 12

_CACHE = {}


def _plan_groups(cap_lens):
    """Slot lengths (max over cores per round-robin slot) + FFD packing."""
    order = np.argsort(-cap_lens, kind="stable")  # longest first
    slot_len = [int(cap_lens[order[s * N_CORES]]) for s in range(CPC)]
    bins = []  # [rows_used, [slot indices]]
    for s in range(CPC):
        ln = slot_len[s]
        for b in bins:
            if b[0] + ln <= 128:
                b[1].append(s)
                b[0] += ln
                break
        else:
            bins.append([ln, [s]])
    groups = []
    for rows, slots in bins:
        offs = []
        off = 0
        for s in slots:
            offs.append(off)
            off += slot_len[s]
        groups.append({"slots": slots, "offs": offs, "P": rows})
    return order, slot_len, groups


def _patch_act_tables():
    """Pin every activation we use to the natural_log_exp_and_others table
    set so the kernel needs exactly one ACT_TABLE_LOAD (the default
    per-function set choice alternates Exp<->Ln sets, costing ~1.3us per
    reload inside the loop)."""
    import concourse.hw_specs as hw_specs

    if getattr(hw_specs, "_act_tables_pinned", False):
        return
    orig = hw_specs.get_activation_tables

    def pinned(module_arch):
        tables = orig(module_arch)
        keep = "natural_log_exp_and_others"
        if keep in tables:
            shared = tables[keep]
            for name, funcs in tables.items():
                if name != keep:
                    tables[name] = funcs - shared
        return tables

    hw_specs.get_activation_tables = pinned
    import concourse.bacc as bacc_mod
    if getattr(bacc_mod, "get_activation_tables", None) is orig:
        bacc_mod.get_activation_tables = pinned
    hw_specs._act_tables_pinned = True


def _build_program(cap_lens):
    import concourse.bacc as bacc
    import concourse.mybir as mybir
    from concourse.tile import TileContext

    _patch_act_tables()

    fp32 = mybir.dt.float32
    f32r = mybir.dt.float32r
    bf16 = mybir.dt.bfloat16
    ACT = mybir.ActivationFunctionType
    ALU = mybir.AluOpType
    AX = mybir.AxisListType

    order, slot_len, groups = _plan_groups(cap_lens)
    NG = len(groups)

    nc = bacc.Bacc("TRN2", target_bir_lowering=False, debug=False,
                   num_devices=N_CORES)

    imagesT_d = nc.dram_tensor("imagesT", [2, 128, IR], f32r, kind="ExternalInput")
    n1sq_d = nc.dram_tensor("n1sq", [CPC, IR], fp32, kind="ExternalInput")
    recT_d, G_d, colsum_d, bcast_d, capmask_d = [], [], [], [], []
    for g, gr in enumerate(groups):
        P = gr["P"]
        ncg = len(gr["slots"])
        recT_d.append(nc.dram_tensor(f"recT{g}", [2, 128, P], f32r,
                                     kind="ExternalInput"))
        G_d.append(nc.dram_tensor(f"G{g}", [P, P], bf16, kind="ExternalInput"))
        colsum_d.append(nc.dram_tensor(f"cs{g}", [P, ncg], bf16,
                                       kind="ExternalInput"))
        bcast_d.append(nc.dram_tensor(f"bc{g}", [CPC, P], bf16,
                                      kind="ExternalInput"))
        capmask_d.append(nc.dram_tensor(f"cm{g}", [P, CPC], bf16,
                                        kind="ExternalInput"))
    out_d = nc.dram_tensor("scores", [CPC, I], fp32, kind="ExternalOutput")

    with TileContext(nc) as tc:
        with (
            tc.tile_pool(name="const", bufs=1) as cpool,
            tc.tile_pool(name="work", bufs=3) as wpool,
            tc.tile_pool(name="small", bufs=2) as spool,
            tc.tile_pool(name="psA", bufs=2, space="PSUM") as psA,
            tc.tile_pool(name="psF", bufs=2, space="PSUM") as psF,
            tc.tile_pool(name="psS", bufs=1, space="PSUM") as psS,
            tc.tile_pool(name="psAcc", bufs=1, space="PSUM") as psAcc,
        ):
            # ---- resident constants ----
            imgT = cpool.tile([128, 2 * IR], f32r, tag="imgT")
            for kc in range(2):
                nc.gpsimd.dma_start(out=imgT[:, kc * IR:(kc + 1) * IR],
                                    in_=imagesT_d[kc, :, :])
            n1sq = cpool.tile([CPC, IR], fp32, tag="n1sq")
            nc.gpsimd.dma_start(out=n1sq[:, :], in_=n1sq_d[:, :])
            recT, Gt, cst, bct, cmt = [], [], [], [], []
            for g, gr in enumerate(groups):
                P = gr["P"]
                ncg = len(gr["slots"])
                rt = cpool.tile([128, 2 * P], f32r, tag=f"recT{g}")
                for kc in range(2):
                    nc.gpsimd.dma_start(out=rt[:, kc * P:(kc + 1) * P],
                                        in_=recT_d[g][kc, :, :])
                recT.append(rt)
                gt = cpool.tile([P, P], bf16, tag=f"G{g}")
                nc.gpsimd.dma_start(out=gt[:, :], in_=G_d[g][:, :])
                Gt.append(gt)
                ct = cpool.tile([P, ncg], bf16, tag=f"cs{g}")
                nc.gpsimd.dma_start(out=ct[:, :], in_=colsum_d[g][:, :])
                cst.append(ct)
                bt = cpool.tile([CPC, P], bf16, tag=f"bc{g}")
                nc.gpsimd.dma_start(out=bt[:, :], in_=bcast_d[g][:, :])
                bct.append(bt)
                mt = cpool.tile([P, CPC], bf16, tag=f"cm{g}")
                nc.gpsimd.dma_start(out=mt[:, :], in_=capmask_d[g][:, :])
                cmt.append(mt)
            scores = cpool.tile([CPC, I], fp32, tag="scores")

            for j in range(N_CHUNKS):
                j0 = j * CHUNK
                num_ps = psAcc.tile([CPC, CHUNK], fp32, tag="num")
                u2_ps = psAcc.tile([CPC, CHUNK], fp32, tag="u2")
                n_ps = psS.tile([CPC, CHUNK], fp32, tag="nrm2")
                Ls, As = [], []
                # pass 1: raw attention, leaky, per-caption norms (batched)
                for g, gr in enumerate(groups):
                    P = gr["P"]
                    first, last = g == 0, g == NG - 1
                    # A_raw = recT.T @ imagesT  (f32r)
                    A_ps = psA.tile([128, CHUNK], fp32, tag="A")
                    for kc in range(2):
                        nc.tensor.matmul(
                            A_ps[:P, :],
                            recT[g][:, kc * P:(kc + 1) * P],
                            imgT[:, kc * IR + j0:kc * IR + j0 + CHUNK],
                            start=(kc == 0), stop=(kc == 1))
                    # leaky: L = prelu(A, alpha=0.1)
                    L = wpool.tile([128, CHUNK], fp32, tag=f"L{g}")
                    nc.scalar.activation(L[:P, :], A_ps[:P, :], ACT.Prelu,
                                         alpha=0.1)
                    Ls.append(L)
                    # keep A in SBUF (bf16) for the num-product later
                    A_sb = wpool.tile([128, CHUNK], bf16, tag=f"Asb{g}")
                    nc.any.tensor_copy(A_sb[:P, :], A_ps[:P, :])
                    As.append(A_sb)
                    # Lsq (bf16; gpsimd works SBUF-only)
                    Lsq = wpool.tile([128, CHUNK], bf16, tag="Lsq")
                    nc.gpsimd.tensor_mul(Lsq[:P, :], L[:P, :], L[:P, :])
                    # nrm2 (all captions, batched): += capmask.T @ Lsq
                    nc.tensor.matmul(n_ps[:, :], cmt[g][:, :], Lsq[:P, :],
                                     start=first, stop=last)
                # rinv9 = 9/sqrt(nrm2) = exp(-0.5*ln(nrm2/81)) for all captions
                # (bf16: per-(cap,ir) scale error cancels in the cosine)
                lnt = spool.tile([CPC, CHUNK], fp32, tag="lnt")
                nc.scalar.activation(lnt[:, :], n_ps[:, :], ACT.Ln,
                                     scale=1.0 / 81.0)
                rinv = spool.tile([CPC, CHUNK], bf16, tag="rinv")
                nc.scalar.activation(rinv[:, :], lnt[:, :], ACT.Exp, scale=-0.5)
                # pass 2: softmax numerator, Gram products, reductions
                for g, gr in enumerate(groups):
                    P = gr["P"]
                    first, last = g == 0, g == NG - 1
                    # broadcast rinv9 to word rows (slot-selector matmul)
                    rb_ps = psS.tile([128, CHUNK], fp32, tag="rb")
                    nc.tensor.matmul(rb_ps[:P, :], bct[g][:, :], rinv[:, :],
                                     start=True, stop=True)
                    # T = L * rinv_b ; E = exp(T) (bf16)
                    T = wpool.tile([128, CHUNK], fp32, tag="T")
                    nc.vector.tensor_mul(T[:P, :], Ls[g][:P, :], rb_ps[:P, :])
                    E = wpool.tile([128, CHUNK], bf16, tag="E")
                    nc.scalar.activation(E[:P, :], T[:P, :], ACT.Exp)
                    # F = G @ E
                    F_ps = psF.tile([128, CHUNK], fp32, tag="F")
                    nc.tensor.matmul(F_ps[:P, :], Gt[g][:, :], E[:P, :],
                                     start=True, stop=True)
                    # P1 = E*A (bf16 2x mode), P2 = E*F
                    P1 = wpool.tile([128, CHUNK], bf16, tag="P1")
                    nc.vector.tensor_mul(P1[:P, :], E[:P, :], As[g][:P, :])
                    P2 = wpool.tile([128, CHUNK], bf16, tag="P2")
                    nc.vector.tensor_mul(P2[:P, :], E[:P, :], F_ps[:P, :])
                    # num += capmask.T @ P1 ; u2 += capmask.T @ P2
                    nc.tensor.matmul(num_ps[:, :], cmt[g][:, :], P1[:P, :],
                                     start=first, stop=last)
                    nc.tensor.matmul(u2_ps[:, :], cmt[g][:, :], P2[:P, :],
                                     start=first, stop=last)
                # rs = num * rsqrt(u2 * n1sq) ; sumexp_r exp(6*rs) per image
                w_t = spool.tile([CPC, CHUNK], fp32, tag="w")
                nc.vector.tensor_mul(w_t[:, :], u2_ps[:, :],
                                     n1sq[:, j0:j0 + CHUNK])
                lw_t = spool.tile([CPC, CHUNK], fp32, tag="lw")
                nc.scalar.activation(lw_t[:, :], w_t[:, :], ACT.Ln)
                q_t = spool.tile([CPC, CHUNK], fp32, tag="q")
                nc.scalar.activation(q_t[:, :], lw_t[:, :], ACT.Exp, scale=-0.5)
                rs_t = spool.tile([CPC, CHUNK], fp32, tag="rs")
                nc.vector.tensor_mul(rs_t[:, :], num_ps[:, :], q_t[:, :])
                e6_t = spool.tile([CPC, CHUNK], fp32, tag="e6")
                nc.scalar.activation(e6_t[:, :], rs_t[:, :], ACT.Exp, scale=6.0)
                nc.vector.tensor_reduce(
                    scores[:, j * IMG_PER_CHUNK:(j + 1) * IMG_PER_CHUNK],
                    e6_t[:, :].rearrange("p (i r) -> p i r", r=R),
                    axis=AX.X, op=ALU.add)
            nc.gpsimd.dma_start(out=out_d[:, :], in_=scores[:, :])
    nc.compile()
    return nc, order, slot_len, groups


def _host_inputs(images, recipes, cap_lens, order, slot_len, groups):
    import ml_dtypes

    imgf = images.reshape(IR, D).astype(np.float32)
    imagesT = np.ascontiguousarray(imgf.T).reshape(2, 128, IR)
    n1sq = (imgf.astype(np.float64) ** 2).sum(axis=1).astype(np.float32)
    n1sq_rep = np.ascontiguousarray(np.broadcast_to(n1sq, (CPC, IR)))

    in_maps = []
    for k in range(N_CORES):
        m = {"imagesT": imagesT, "n1sq": n1sq_rep}
        for g, gr in enumerate(groups):
            P = gr["P"]
            ncg = len(gr["slots"])
            Rg = np.zeros((P, D), np.float32)  # packed, zero-padded recipes
            cs = np.zeros((P, ncg), np.float32)
            bc = np.zeros((CPC, P), np.float32)
            cm = np.zeros((P, CPC), np.float32)
            Gm = np.zeros((P, P), np.float32)
            for li, (s, off) in enumerate(zip(gr["slots"], gr["offs"])):
                cap = int(order[s * N_CORES + k])
                ln = int(cap_lens[cap])
                lp = slot_len[s]
                rws = recipes[cap, :ln, :].astype(np.float32)
                Rg[off:off + ln, :] = rws
                Gm[off:off + ln, off:off + ln] = rws @ rws.T
                cs[off:off + lp, li] = 1.0
                bc[s, off:off + lp] = 1.0
                cm[off:off + lp, s] = 1.0
            m[f"recT{g}"] = np.ascontiguousarray(Rg.T).reshape(2, 128, P)
            m[f"G{g}"] = Gm.astype(ml_dtypes.bfloat16)
            m[f"cs{g}"] = cs.astype(ml_dtypes.bfloat16)
            m[f"bc{g}"] = bc.astype(ml_dtypes.bfloat16)
            m[f"cm{g}"] = cm.astype(ml_dtypes.bfloat16)
        in_maps.append(m)
    return in_maps


def run_sharded(images, recipes, cap_lens, **spmd_kwargs):
    """Compile (cached), run on 8 cores, return (sumexp (C, I) fp64, results)."""
    from concourse.bass_utils import run_bass_kernel_spmd

    cap_lens = np.asarray(cap_lens).astype(np.int32)
    key = cap_lens.tobytes()
    if key not in _CACHE:
        _CACHE[key] = _build_program(cap_lens)
    nc, order, slot_len, groups = _CACHE[key]

    in_maps = _host_inputs(np.asarray(images), np.asarray(recipes), cap_lens,
                           order, slot_len, groups)
    res = run_bass_kernel_spmd(nc, in_maps, list(range(N_CORES)), **spmd_kwargs)

    sumexp = np.zeros((C, I), np.float64)
    for k in range(N_CORES):
        sc = res.results[k]["scores"].astype(np.float64)  # [CPC, I]
        for s in range(CPC):
            sumexp[int(order[s * N_CORES + k])] = sc[s]
    return sumexp, res


def kernel(images, recipes, cap_lens):
    sumexp, _ = run_sharded(images, recipes, cap_lens)
    S = (np.log(sumexp) / 6.0).T  # (I, C)
    diag = np.diag(S)
    eye = np.eye(I, dtype=bool)
    ci = np.where(eye, 0.0, np.maximum(0.2 + S - diag[None, :], 0.0))
    cr = np.where(eye, 0.0, np.maximum(0.2 + S - diag[:, None], 0.0))
    return np.float32(ci.sum() + cr.sum())


# revision 12
# speedup vs baseline: 1.6422x; 1.0201x over previous
"""Trainium2 Bass kernel for the SCAN cross-attention contrastive loss.

Math (validated against the reference, exact in fp64):
  For caption c with words zeroed beyond its true length:
    A[w, ir]   = <recipes[c, w, :], images_flat[ir, :]>           (raw attention)
    L          = leaky_relu(A, 0.1)       (zero rows stay zero)
    rinv9[ir]  = 9 / ||L[:, ir]||
    E          = exp(L * rinv9)           (softmax numerator; the denominator
                                           cancels in the cosine below)
    num[ir]    = sum_w E * A              (invalid rows: E*0 = 0)
    u2[ir]     = sum_w E * (G @ E),  G = R_c R_c^T  (Gram; zero rows kill pad E)
    rs[ir]     = num / (||img_ir|| * sqrt(u2))      (= row_sim of the reference)
    score[c,i] = sum_{r in image i} exp(6 * rs)     (log/6 applied on host)
  Final hinge-loss reduction over the 96x96 score matrix is done on host (tiny).

Sharding: captions sorted by length, dealt round-robin to 8 cores (slot s of
core k = sorted[s*8+k]) so every core shares the same per-slot padded length,
then slots are FFD-bin-packed into <=128-partition groups. One SPMD program;
per-core data (packed recipes, Gram blocks, masks) differs.
"""

import sys

sys.path.insert(0, "/opt/trn_rl_repo")

import numpy as np

I, R, D = 96, 36, 256
C, W = 96, 48
IR = I * R  # 3456
N_CORES = 8
CPC = C // N_CORES  # captions per core = 12
CHUNK = 432  # IR columns per iteration = 12 images = one PSUM bank
N_CHUNKS = IR // CHUNK  # 8
IMG_PER_CHUNK = CHUNK // R  # 12

_CACHE = {}


def _plan_groups(cap_lens):
    """Slot lengths (max over cores per round-robin slot) + FFD packing."""
    order = np.argsort(-cap_lens, kind="stable")  # longest first
    slot_len = [int(cap_lens[order[s * N_CORES]]) for s in range(CPC)]
    bins = []  # [rows_used, [slot indices]]
    for s in range(CPC):
        ln = slot_len[s]
        for b in bins:
            if b[0] + ln <= 128:
                b[1].append(s)
                b[0] += ln
                break
        else:
            bins.append([ln, [s]])
    groups = []
    for rows, slots in bins:
        offs = []
        off = 0
        for s in slots:
            offs.append(off)
            off += slot_len[s]
        groups.append({"slots": slots, "offs": offs, "P": rows})
    return order, slot_len, groups


def _patch_act_tables():
    """Pin every activation we use to the natural_log_exp_and_others table
    set so the kernel needs exactly one ACT_TABLE_LOAD (the default
    per-function set choice alternates Exp<->Ln sets, costing ~1.3us per
    reload inside the loop)."""
    import concourse.hw_specs as hw_specs

    if getattr(hw_specs, "_act_tables_pinned", False):
        return
    orig = hw_specs.get_activation_tables

    def pinned(module_arch):
        tables = orig(module_arch)
        keep = "natural_log_exp_and_others"
        if keep in tables:
            shared = tables[keep]
            for name, funcs in tables.items():
                if name != keep:
                    tables[name] = funcs - shared
        return tables

    hw_specs.get_activation_tables = pinned
    import concourse.bacc as bacc_mod
    if getattr(bacc_mod, "get_activation_tables", None) is orig:
        bacc_mod.get_activation_tables = pinned
    hw_specs._act_tables_pinned = True


def _build_program(cap_lens):
    import concourse.bacc as bacc
    import concourse.mybir as mybir
    from concourse.tile import TileContext

    _patch_act_tables()

    fp32 = mybir.dt.float32
    f32r = mybir.dt.float32r
    bf16 = mybir.dt.bfloat16
    ACT = mybir.ActivationFunctionType
    ALU = mybir.AluOpType
    AX = mybir.AxisListType

    order, slot_len, groups = _plan_groups(cap_lens)
    NG = len(groups)

    nc = bacc.Bacc("TRN2", target_bir_lowering=False, debug=False,
                   num_devices=N_CORES)

    imagesT_d = nc.dram_tensor("imagesT", [2, 128, IR], f32r, kind="ExternalInput")
    n1sq_d = nc.dram_tensor("n1sq", [CPC, IR], fp32, kind="ExternalInput")
    recT_d, G_d, colsum_d, bcast_d, capmask_d = [], [], [], [], []
    for g, gr in enumerate(groups):
        P = gr["P"]
        ncg = len(gr["slots"])
        recT_d.append(nc.dram_tensor(f"recT{g}", [2, 128, P], f32r,
                                     kind="ExternalInput"))
        G_d.append(nc.dram_tensor(f"G{g}", [P, P], bf16, kind="ExternalInput"))
        colsum_d.append(nc.dram_tensor(f"cs{g}", [P, ncg], bf16,
                                       kind="ExternalInput"))
        bcast_d.append(nc.dram_tensor(f"bc{g}", [CPC, P], bf16,
                                      kind="ExternalInput"))
        capmask_d.append(nc.dram_tensor(f"cm{g}", [P, CPC], bf16,
                                        kind="ExternalInput"))
    out_d = nc.dram_tensor("scores", [CPC, I], fp32, kind="ExternalOutput")

    with TileContext(nc) as tc:
        with (
            tc.tile_pool(name="const", bufs=1) as cpool,
            tc.tile_pool(name="work", bufs=4) as wpool,
            tc.tile_pool(name="small", bufs=2) as spool,
            tc.tile_pool(name="psA", bufs=2, space="PSUM") as psA,
            tc.tile_pool(name="psF", bufs=2, space="PSUM") as psF,
            tc.tile_pool(name="psS", bufs=1, space="PSUM") as psS,
            tc.tile_pool(name="psAcc", bufs=1, space="PSUM") as psAcc,
        ):
            # ---- resident constants ----
            imgT = cpool.tile([128, 2 * IR], f32r, tag="imgT")
            for kc in range(2):
                nc.gpsimd.dma_start(out=imgT[:, kc * IR:(kc + 1) * IR],
                                    in_=imagesT_d[kc, :, :])
            n1sq = cpool.tile([CPC, IR], fp32, tag="n1sq")
            nc.gpsimd.dma_start(out=n1sq[:, :], in_=n1sq_d[:, :])
            recT, Gt, cst, bct, cmt = [], [], [], [], []
            for g, gr in enumerate(groups):
                P = gr["P"]
                ncg = len(gr["slots"])
                rt = cpool.tile([128, 2 * P], f32r, tag=f"recT{g}")
                for kc in range(2):
                    nc.gpsimd.dma_start(out=rt[:, kc * P:(kc + 1) * P],
                                        in_=recT_d[g][kc, :, :])
                recT.append(rt)
                gt = cpool.tile([P, P], bf16, tag=f"G{g}")
                nc.gpsimd.dma_start(out=gt[:, :], in_=G_d[g][:, :])
                Gt.append(gt)
                ct = cpool.tile([P, ncg], bf16, tag=f"cs{g}")
                nc.gpsimd.dma_start(out=ct[:, :], in_=colsum_d[g][:, :])
                cst.append(ct)
                bt = cpool.tile([CPC, P], bf16, tag=f"bc{g}")
                nc.gpsimd.dma_start(out=bt[:, :], in_=bcast_d[g][:, :])
                bct.append(bt)
                mt = cpool.tile([P, CPC], bf16, tag=f"cm{g}")
                nc.gpsimd.dma_start(out=mt[:, :], in_=capmask_d[g][:, :])
                cmt.append(mt)
            scores = cpool.tile([CPC, I], fp32, tag="scores")

            for j in range(N_CHUNKS):
                j0 = j * CHUNK
                num_ps = psAcc.tile([CPC, CHUNK], fp32, tag="num")
                u2_ps = psAcc.tile([CPC, CHUNK], fp32, tag="u2")
                n_ps = psS.tile([CPC, CHUNK], fp32, tag="nrm2")
                Ls, As = [], []
                # pass 1: raw attention, leaky, per-caption norms (batched)
                for g, gr in enumerate(groups):
                    P = gr["P"]
                    first, last = g == 0, g == NG - 1
                    # A_raw = recT.T @ imagesT  (f32r)
                    A_ps = psA.tile([128, CHUNK], fp32, tag="A")
                    for kc in range(2):
                        nc.tensor.matmul(
                            A_ps[:P, :],
                            recT[g][:, kc * P:(kc + 1) * P],
                            imgT[:, kc * IR + j0:kc * IR + j0 + CHUNK],
                            start=(kc == 0), stop=(kc == 1))
                    # leaky: L = prelu(A, alpha=0.1)
                    L = wpool.tile([128, CHUNK], fp32, tag=f"L{g}")
                    nc.scalar.activation(L[:P, :], A_ps[:P, :], ACT.Prelu,
                                         alpha=0.1)
                    Ls.append(L)
                    # keep A in SBUF (bf16) for the num-product later
                    A_sb = wpool.tile([128, CHUNK], bf16, tag=f"Asb{g}")
                    nc.any.tensor_copy(A_sb[:P, :], A_ps[:P, :])
                    As.append(A_sb)
                    # Lsq (bf16; gpsimd works SBUF-only)
                    Lsq = wpool.tile([128, CHUNK], bf16, tag="Lsq")
                    nc.gpsimd.tensor_mul(Lsq[:P, :], L[:P, :], L[:P, :])
                    # nrm2 (all captions, batched): += capmask.T @ Lsq
                    nc.tensor.matmul(n_ps[:, :], cmt[g][:, :], Lsq[:P, :],
                                     start=first, stop=last)
                # rinv9 = 9/sqrt(nrm2) = exp(-0.5*ln(nrm2/81)) for all captions
                # (bf16: per-(cap,ir) scale error cancels in the cosine)
                lnt = spool.tile([CPC, CHUNK], fp32, tag="lnt")
                nc.scalar.activation(lnt[:, :], n_ps[:, :], ACT.Ln,
                                     scale=1.0 / 81.0)
                rinv = spool.tile([CPC, CHUNK], bf16, tag="rinv")
                nc.scalar.activation(rinv[:, :], lnt[:, :], ACT.Exp, scale=-0.5)
                # pass 2: softmax numerator, Gram products, reductions
                for g, gr in enumerate(groups):
                    P = gr["P"]
                    first, last = g == 0, g == NG - 1
                    # broadcast rinv9 to word rows (slot-selector matmul)
                    rb_ps = psS.tile([128, CHUNK], fp32, tag="rb")
                    nc.tensor.matmul(rb_ps[:P, :], bct[g][:, :], rinv[:, :],
                                     start=True, stop=True)
                    # T = L * rinv_b ; E = exp(T) (bf16)
                    T = wpool.tile([128, CHUNK], fp32, tag="T")
                    nc.vector.tensor_mul(T[:P, :], Ls[g][:P, :], rb_ps[:P, :])
                    E = wpool.tile([128, CHUNK], bf16, tag="E")
                    nc.scalar.activation(E[:P, :], T[:P, :], ACT.Exp)
                    # F = G @ E
                    F_ps = psF.tile([128, CHUNK], fp32, tag="F")
                    nc.tensor.matmul(F_ps[:P, :], Gt[g][:, :], E[:P, :],
                                     start=True, stop=True)
                    # P1 = E*A (bf16 2x mode), P2 = E*F
                    P1 = wpool.tile([128, CHUNK], bf16, tag="P1")
                    nc.vector.tensor_mul(P1[:P, :], E[:P, :], As[g][:P, :])
                    P2 = wpool.tile([128, CHUNK], bf16, tag="P2")
                    nc.vector.tensor_mul(P2[:P, :], E[:P, :], F_ps[:P, :])
                    # num += capmask.T @ P1 ; u2 += capmask.T @ P2
                    nc.tensor.matmul(num_ps[:, :], cmt[g][:, :], P1[:P, :],
                                     start=first, stop=last)
                    nc.tensor.matmul(u2_ps[:, :], cmt[g][:, :], P2[:P, :],
                                     start=first, stop=last)
                # rs = num * rsqrt(u2 * n1sq) ; sumexp_r exp(6*rs) per image
                w_t = spool.tile([CPC, CHUNK], fp32, tag="w")
                nc.vector.tensor_mul(w_t[:, :], u2_ps[:, :],
                                     n1sq[:, j0:j0 + CHUNK])
                lw_t = spool.tile([CPC, CHUNK], fp32, tag="lw")
                nc.scalar.activation(lw_t[:, :], w_t[:, :], ACT.Ln)
                q_t = spool.tile([CPC, CHUNK], fp32, tag="q")
                nc.scalar.activation(q_t[:, :], lw_t[:, :], ACT.Exp, scale=-0.5)
                rs_t = spool.tile([CPC, CHUNK], fp32, tag="rs")
                nc.vector.tensor_mul(rs_t[:, :], num_ps[:, :], q_t[:, :])
                e6_t = spool.tile([CPC, CHUNK], fp32, tag="e6")
                nc.scalar.activation(e6_t[:, :], rs_t[:, :], ACT.Exp, scale=6.0)
                nc.vector.tensor_reduce(
                    scores[:, j * IMG_PER_CHUNK:(j + 1) * IMG_PER_CHUNK],
                    e6_t[:, :].rearrange("p (i r) -> p i r", r=R),
                    axis=AX.X, op=ALU.add)
            nc.gpsimd.dma_start(out=out_d[:, :], in_=scores[:, :])
    nc.compile()
    return nc, order, slot_len, groups


def _host_inputs(images, recipes, cap_lens, order, slot_len, groups):
    import ml_dtypes

    imgf = images.reshape(IR, D).astype(np.float32)
    imagesT = np.ascontiguousarray(imgf.T).reshape(2, 128, IR)
    n1sq = (imgf.astype(np.float64) ** 2).sum(axis=1).astype(np.float32)
    n1sq_rep = np.ascontiguousarray(np.broadcast_to(n1sq, (CPC, IR)))

    in_maps = []
    for k in range(N_CORES):
        m = {"imagesT": imagesT, "n1sq": n1sq_rep}
        for g, gr in enumerate(groups):
            P = gr["P"]
            ncg = len(gr["slots"])
            Rg = np.zeros((P, D), np.float32)  # packed, zero-padded recipes
            cs = np.zeros((P, ncg), np.float32)
            bc = np.zeros((CPC, P), np.float32)
            cm = np.zeros((P, CPC), np.float32)
            Gm = np.zeros((P, P), np.float32)
            for li, (s, off) in enumerate(zip(gr["slots"], gr["offs"])):
                cap = int(order[s * N_CORES + k])
                ln = int(cap_lens[cap])
                lp = slot_len[s]
                rws = recipes[cap, :ln, :].astype(np.float32)
                Rg[off:off + ln, :] = rws
                Gm[off:off + ln, off:off + ln] = rws @ rws.T
                cs[off:off + lp, li] = 1.0
                bc[s, off:off + lp] = 1.0
                cm[off:off + lp, s] = 1.0
            m[f"recT{g}"] = np.ascontiguousarray(Rg.T).reshape(2, 128, P)
            m[f"G{g}"] = Gm.astype(ml_dtypes.bfloat16)
            m[f"cs{g}"] = cs.astype(ml_dtypes.bfloat16)
            m[f"bc{g}"] = bc.astype(ml_dtypes.bfloat16)
            m[f"cm{g}"] = cm.astype(ml_dtypes.bfloat16)
        in_maps.append(m)
    return in_maps


def run_sharded(images, recipes, cap_lens, **spmd_kwargs):
    """Compile (cached), run on 8 cores, return (sumexp (C, I) fp64, results)."""
    from concourse.bass_utils import run_bass_kernel_spmd

    cap_lens = np.asarray(cap_lens).astype(np.int32)
    key = cap_lens.tobytes()
    if key not in _CACHE:
        _CACHE[key] = _build_program(cap_lens)
    nc, order, slot_len, groups = _CACHE[key]

    in_maps = _host_inputs(np.asarray(images), np.asarray(recipes), cap_lens,
                           order, slot_len, groups)
    res = run_bass_kernel_spmd(nc, in_maps, list(range(N_CORES)), **spmd_kwargs)

    sumexp = np.zeros((C, I), np.float64)
    for k in range(N_CORES):
        sc = res.results[k]["scores"].astype(np.float64)  # [CPC, I]
        for s in range(CPC):
            sumexp[int(order[s * N_CORES + k])] = sc[s]
    return sumexp, res


def kernel(images, recipes, cap_lens):
    sumexp, _ = run_sharded(images, recipes, cap_lens)
    S = (np.log(sumexp) / 6.0).T  # (I, C)
    diag = np.diag(S)
    eye = np.eye(I, dtype=bool)
    ci = np.where(eye, 0.0, np.maximum(0.2 + S - diag[None, :], 0.0))
    cr = np.where(eye, 0.0, np.maximum(0.2 + S - diag[:, None], 0.0))
    return np.float32(ci.sum() + cr.sum())
